# revision 34
# baseline (speedup 1.0000x reference)
"""ClassCapsule dynamic-routing kernel for 8 Trainium2 NeuronCores.

Problem (hardcoded shapes):
    x:    [64, 2048, 16]  fp32
    W:    [2048, 16, 1024] fp32
    bias: [64, 16]        fp32
    out:  [64, 64, 16]    fp32  (squeezed v after 3 routing iterations)

Strategy (in_caps-sharded, W resident in SBUF, u_hat recomputed per
iteration, per-iteration AllReduce of the small s tensor):
  - in_caps=2048 split across 8 cores (256 each); every core holds the
    full batch B=64.  W slice (bf16) lives in SBUF for the whole kernel,
    so u_hat is recomputed on the PE each routing iteration instead of
    being bounced through DRAM.  Total HBM traffic is ~12 MB/core.
  - u_hat tiles [128=(i4,b32), 1024=(d16,n64)] come from 4-way
    block-diagonal matmuls: lhsT [K=64=(i4,e16), M=128=(i4,b32)], two
    blocks packed across the 128 K-partitions (PE row strips at base
    partitions 0/64).  Column order (d major, n minor) keeps the
    free-dim broadcast of c packed so DVE runs in 2x bf16 mode.
  - iteration 0 (uniform c): s0 = sum_i u/64 collapses into a dense
    x^T @ W matmul over K=128 - no u_hat materialization at all.
  - routing: agreement = u*v reduced over d via a halving add tree
    (DVE, bf16), softmax over n (ACT exp + DVE), weighted sum over i
    via selector matmuls on the PE accumulating in PSUM.
  - s [64,1024] partials are AllReduced (collective_compute, bf16)
    across the 8 cores each iteration; squash/v computed redundantly.
"""

import numpy as np
import ml_dtypes

import concourse.bass as bass
import concourse.tile as tile
from concourse import bacc, mybir
from concourse.bass_utils import run_bass_kernel_spmd

# ---------------------------------------------------------------- constants
B, IC, E = 64, 2048, 16          # batch, in_caps, in_dim
NCAP, D = 64, 16                 # n_caps, cap_dim
ND = NCAP * D                    # 1024
CORES = 8
ICL = IC // CORES                # 256 local in_caps
SG = 32                          # column groups (8 in_caps each)
HB = 32                          # half-batch chunk
EPS = 1e-7

FP = mybir.dt.float32
BF = mybir.dt.bfloat16
BF_NP = ml_dtypes.bfloat16


def _host_prep(x, W, bias):
    """Per-core host-side tensors (bf16, (d,n) column order).

    Column group gg covers local in_caps i = 8*gg + 4*s + l  (strip s in
    {0,1} at partition base 64*s, lane l in 0..3).  Partition row layout
    for W / x lhsT: p = (s2, l4, e16)."""
    # W columns reordered from (n,d) to (d,n): new_col = d*64 + n
    W_dn = W.reshape(IC, E, NCAP, D).transpose(0, 1, 3, 2).reshape(IC, E, ND)

    w_all, xbd_all, xd_all = [], [], []
    for c in range(CORES):
        sl = slice(c * ICL, (c + 1) * ICL)
        W_c = W_dn[sl]                                   # [256, 16, 1024]
        # [gg, s, l, e, nd] -> [(s l e)=128, gg, nd]
        w_all.append(np.ascontiguousarray(
            W_c.reshape(SG, 2, 4, E, ND).transpose(1, 2, 3, 0, 4)
            .reshape(128, SG, ND)).astype(BF_NP))

        x_c = x[:, sl]                                   # [64, 256, 16]
        x_r = x_c.reshape(2, HB, SG, 2, 4, E).transpose(3, 4, 5, 2, 0, 1)
        # x_r: [s, l, e, gg, ch, b']
        # 4-way block-diag lhsT: [(s l e)=128, gg, ch, (l' b32)=128]
        arr = np.zeros((2, 4, E, SG, 2, 4, HB), dtype=np.float32)
        for l in range(4):
            arr[:, l, :, :, :, l, :] = x_r[:, l]
        xbd_all.append(arr.reshape(128, SG, 2, 128).astype(BF_NP))

        # dense lhsT for iter-0 direct sum: [(s l e)=128, gg, m=b pad 128]
        xd = np.zeros((128, SG, 128), dtype=np.float32)
        xd[:, :, :B] = x_c.reshape(B, SG, 2, 4, E).transpose(2, 3, 4, 1, 0) \
            .reshape(128, SG, B)
        xd_all.append(xd.astype(BF_NP))

    # selectors, one per batch chunk: sel[ch][k=(l4,b32), m=ch*32+(k%32)]
    sels = np.zeros((2, 128, 128), dtype=np.float32)
    for ch in range(2):
        k = np.arange(128)
        sels[ch, k, ch * HB + (k % HB)] = 1.0
    sels = np.ascontiguousarray(sels.transpose(1, 0, 2)).astype(BF_NP)

    # bias in (d,n) order, tiled over batch: [64, 1024]
    bias_dn = np.ascontiguousarray(bias.T).reshape(1, ND)       # [d,n] flat
    bias_f = np.tile(bias_dn, (B, 1)).astype(np.float32)
    return w_all, xbd_all, xd_all, sels, bias_f


def _build_program():
    nc = bacc.Bacc("TRN2", target_bir_lowering=False, num_devices=CORES)

    w_d = nc.dram_tensor("w_d", [128, SG, ND], BF, kind="ExternalInput")
    xbd_d = nc.dram_tensor("xbd_d", [128, SG, 2, 128], BF, kind="ExternalInput")
    xd_d = nc.dram_tensor("xd_d", [128, SG, 128], BF, kind="ExternalInput")
    sel_d = nc.dram_tensor("sel_d", [128, 2, 128], BF, kind="ExternalInput")
    bias_d = nc.dram_tensor("bias_d", [B, ND], FP, kind="ExternalInput")
    v_out = nc.dram_tensor("v_out", [B, ND], FP, kind="ExternalOutput")

    v_scr = nc.dram_tensor("v_scr", [B, ND], BF)     # bounce for vb build

    with tile.TileContext(nc) as tc:
        with (
            tc.tile_pool(name="consts", bufs=1) as cp,
            tc.tile_pool(name="ubf", bufs=5) as up,       # [128, 4096] grouped
            tc.tile_pool(name="tmp", bufs=3) as tp,       # [128, 4096] tmp+ws ring
            tc.tile_pool(name="vb", bufs=1) as vbp,
            tc.tile_pool(name="smalls", bufs=2) as sp,
            tc.tile_pool(name="sq", bufs=1) as qp,
            tc.tile_pool(name="ups", bufs=3, space="PSUM") as psp,
            tc.tile_pool(name="sps", bufs=1, space="PSUM") as psa,
            tc.tile_pool(name="bstate", bufs=1) as bsp,
            tc.tile_pool(name="dram", bufs=2, space="DRAM") as dp,
        ):
            # ---- resident tensors.  Load order matters: iter 0 needs xd +
            # w chunks; xbd is only needed at iter 1 so it loads last.
            xd_sb = up.tile([128, SG * 128], BF, tag="u_g")
            nc.sync.dma_start(out=xd_sb, in_=xd_d[:, :, :])
            sel_sb = cp.tile([128, 2 * 128], BF)
            nc.sync.dma_start(out=sel_sb, in_=sel_d[:, :, :])
            bias_sb = cp.tile([B, ND], FP)
            nc.sync.dma_start(out=bias_sb, in_=bias_d[:, :])
            eps_t = cp.tile([B, 1], FP)
            nc.vector.memset(eps_t, EPS)
            w_sb = cp.tile([128, SG * ND], BF)
            WCH = 4  # groups per load chunk; per-chunk deps let iter0 start early
            for chk in range(SG // WCH):
                nc.sync.dma_start(
                    out=w_sb[:, chk * WCH * ND:(chk + 1) * WCH * ND],
                    in_=w_d[:, chk * WCH:(chk + 1) * WCH, :])
            xbd_sb = cp.tile([128, SG * 2 * 128], BF)
            nc.sync.dma_start(out=xbd_sb, in_=xbd_d[:, :, :, :])

            # warm up the collective path while inputs stream in: the first
            # AllReduce pays one-time channel setup, so do a tiny dummy one
            warm_in = dp.tile([B, 4], FP, tag="warm_in")
            warm_out = dp.tile([B, 4], FP, tag="warm_out")
            warm_sb = cp.tile([B, 4], FP)
            nc.vector.memset(warm_sb, 0.0)
            nc.sync.dma_start(out=warm_in[:], in_=warm_sb)
            nc.gpsimd.collective_compute(
                "AllReduce",
                mybir.AluOpType.add,
                replica_groups=[list(range(CORES))],
                ins=[warm_in[:].opt()],
                outs=[warm_out[:].opt()],
            )

            # routing logits state: [128=(l4,b32), (gg, s, ch, n)]
            b_all = bsp.tile([128, SG * 4 * NCAP], FP)

            # ---------------- AllReduce s -> (scale,bias) -> squash -> v
            def reduce_squash_v(s_ps, scale, last):
                # AllReduce in bf16: halves the collective payload; the
                # ~0.4% rounding on s is well inside the error budget
                s_par = qp.tile([B, ND], BF, tag="q0")
                nc.scalar.copy(out=s_par, in_=s_ps[0:B, :])
                s_in = dp.tile([B, ND], BF, tag="cc_in")
                nc.sync.dma_start(out=s_in[:], in_=s_par)
                s_red = dp.tile([B, ND], BF, tag="cc_out")
                nc.gpsimd.collective_compute(
                    "AllReduce",
                    mybir.AluOpType.add,
                    replica_groups=[list(range(CORES))],
                    ins=[s_in[:].opt()],
                    outs=[s_red[:].opt()],
                )
                s_glob = qp.tile([B, ND], BF, tag="q2b")
                nc.sync.dma_start(out=s_glob, in_=s_red[:])
                # s = s_glob*scale + bias
                s_sb = qp.tile([B, ND], FP, tag="q1")
                nc.vector.scalar_tensor_tensor(
                    out=s_sb, in0=s_glob, scalar=float(scale), in1=bias_sb,
                    op0=mybir.AluOpType.mult, op1=mybir.AluOpType.add)
                sqr = qp.tile([B, ND], FP, tag="q2")
                nc.scalar.square(out=sqr, in_=s_sb)
                nsq = sp.tile([B, NCAP], FP, tag="nsq")
                nc.vector.reduce_sum(
                    out=nsq, in_=sqr.rearrange("p (d n) -> p n d", d=D),
                    axis=mybir.AxisListType.X)
                norm = sp.tile([B, NCAP], FP, tag="norm")
                nc.scalar.activation(out=norm, in_=nsq,
                                     func=mybir.ActivationFunctionType.Sqrt,
                                     bias=eps_t[:, :], scale=1.0)
                den = sp.tile([B, NCAP], FP, tag="den")
                nc.vector.scalar_tensor_tensor(
                    out=den, in0=nsq, scalar=float(EPS + 1.0), in1=norm,
                    op0=mybir.AluOpType.add, op1=mybir.AluOpType.mult)
                rden = sp.tile([B, NCAP], FP, tag="rden")
                nc.vector.reciprocal(out=rden, in_=den)
                fac = sp.tile([B, NCAP], FP, tag="fac")
                nc.vector.scalar_tensor_tensor(
                    out=fac, in0=nsq, scalar=float(EPS), in1=rden,
                    op0=mybir.AluOpType.add, op1=mybir.AluOpType.mult)
                v_sb = qp.tile([B, ND], FP, tag="q2")
                fac_b = bass.AP(tensor=fac.tensor, offset=fac.offset,
                                ap=[list(fac.ap[0]), [0, D], list(fac.ap[1])])
                nc.vector.tensor_mul(
                    v_sb.rearrange("p (d n) -> p d n", d=D),
                    s_sb.rearrange("p (d n) -> p d n", d=D),
                    fac_b)
                if last:
                    nc.sync.dma_start(out=v_out[:, :], in_=v_sb)
                    return None
                v_bf = qp.tile([B, ND], BF, tag="q0")
                nc.vector.tensor_copy(out=v_bf, in_=v_sb)
                nc.sync.dma_start(out=v_scr[:, :], in_=v_bf)
                # vb[128=(l4,b32), (ch,d,n)]: row (l,b') of ch holds v[ch*32+b']
                vb = vbp.tile([128, 2 * ND], BF, tag="vb")
                for ch in range(2):
                    src = bass.AP(tensor=v_scr, offset=ch * HB * ND,
                                  ap=[[0, 4], [ND, HB], [1, ND]])
                    nc.sync.dma_start(
                        out=vb[:, ch * ND:(ch + 1) * ND], in_=src)
                return vb

            # ================= iter 0: s0 = (1/64) sum_i u  ================
            s_ps = psa.tile([128, ND], FP, tag="s_acc")
            # HAM warm-up: keep the PE busy while W streams in so iter 0
            # runs at full clock.  Garbage results land in s_ps and are
            # discarded by the first real matmul's start=True reset.
            for wm in range(48):
                nc.tensor.matmul(
                    s_ps[:, 0:128], sel_sb[:, 0:128], sel_sb[:, 0:128],
                    start=True, stop=True, skip_group_check=True)
            for g in range(SG):
                for h in range(2):
                    nc.tensor.matmul(
                        s_ps[:, h * 512:(h + 1) * 512],
                        xd_sb[:, g * 128:(g + 1) * 128],
                        w_sb[:, g * ND + h * 512:g * ND + (h + 1) * 512],
                        start=(g == 0), stop=(g == SG - 1))
            vb = reduce_squash_v(s_ps, 1.0 / NCAP, last=False)

            # ================= routing iterations 1 and 2 =================
            # Software-pipelined: stage A(g) = proj + psum copy + u*v +
            # d-reduce tree + exp; stage B(g) = Z/recip/scale + u*c + sel
            # matmuls.  Emitting A(g+1) before B(g) keeps the (in-order)
            # DVE queue fed while ACT's exp and GpSimd's scale for block g
            # are still in flight.
            for it in (1, 2):
                s_ps = psa.tile([128, ND], FP, tag="s_acc")

                def stage_a(g, vb, it=it):
                    u_g = up.tile([128, 4 * ND], BF, tag="u_g")
                    for q in range(4):          # q = (s strip, ch half-batch)
                        s_, ch = q >> 1, q & 1
                        u_ps = psp.tile([128, ND], FP, tag="u_ps")
                        lhs = xbd_sb[64 * s_:64 * (s_ + 1),
                                     (g * 2 + ch) * 128:(g * 2 + ch + 1) * 128]
                        for h in range(2):
                            nc.tensor.matmul(
                                u_ps[:, h * 512:(h + 1) * 512],
                                lhs,
                                w_sb[64 * s_:64 * (s_ + 1),
                                     g * ND + h * 512:g * ND + (h + 1) * 512],
                                start=True, stop=True)
                        nc.scalar.copy(
                            out=u_g[:, q * ND:(q + 1) * ND], in_=u_ps)
                    # tmp = u * v  (bf16, packed -> 2x DVE; v bcast over s)
                    tmp = tp.tile([128, 4 * ND], BF, tag="tmp")
                    vb_b = bass.AP(tensor=vb.tensor, offset=vb.offset,
                                   ap=[list(vb.ap[0]), [0, 2], [ND, 2],
                                       [1, ND]])
                    nc.vector.tensor_mul(
                        tmp.rearrange("p (s c f) -> p s c f", s=2, c=2),
                        u_g.rearrange("p (s c f) -> p s c f", s=2, c=2),
                        vb_b)
                    # reduce over d: halving add tree on [p, q, (d n)]
                    t3 = tmp.rearrange("p (c f) -> p c f", c=4)
                    b_dst = b_all[:, g * 4 * NCAP:(g + 1) * 4 * NCAP]
                    b3 = b_dst.rearrange("p (c n) -> p c n", c=4)
                    for half in (512, 256, 128, 64):
                        src_hi = bass.AP(
                            tensor=tmp.tensor, offset=tmp.offset + half,
                            ap=[list(tmp.ap[0]), [ND, 4], [1, half]])
                        if half > 64:
                            nc.vector.tensor_add(
                                t3[:, :, 0:half], t3[:, :, 0:half], src_hi)
                        else:
                            # final add -> b state (fp32, contiguous)
                            if it == 1:
                                nc.vector.tensor_add(
                                    b3, t3[:, :, 0:64], src_hi)
                            else:
                                agr = sp.tile([128, 4 * NCAP], FP, tag="agr")
                                a3 = agr.rearrange("p (c n) -> p c n", c=4)
                                nc.vector.tensor_add(
                                    a3, t3[:, :, 0:64], src_hi)
                                nc.vector.tensor_add(b_dst, b_dst, agr)
                    # softmax numerator: exp on ACT
                    c_un = sp.tile([128, 4 * NCAP], BF, tag="c_un")
                    nc.scalar.activation(
                        out=c_un, in_=b_dst,
                        func=mybir.ActivationFunctionType.Exp)
                    return u_g, c_un

                def stage_b(g, u_g, c_un):
                    zsum = sp.tile([128, 4], FP, tag="zsum")
                    nc.vector.reduce_sum(
                        out=zsum, in_=c_un.rearrange("p (c n) -> p c n", c=4),
                        axis=mybir.AxisListType.X)
                    rec = sp.tile([128, 4], BF, tag="rec")
                    with nc.allow_low_precision(reason="1/Z in bf16 is fine for softmax scale"):
                        nc.vector.reciprocal(out=rec, in_=zsum)
                    c_bf = sp.tile([128, 4 * NCAP], BF, tag="c_bf")
                    rec_b = bass.AP(tensor=rec.tensor, offset=rec.offset,
                                    ap=[list(rec.ap[0]), [1, 4], [0, NCAP]])
                    nc.gpsimd.tensor_mul(
                        c_bf.rearrange("p (c n) -> p c n", c=4),
                        c_un.rearrange("p (c n) -> p c n", c=4),
                        rec_b)
                    # w = u * c (c bcast over d; last dim packed -> 2x DVE)
                    w_g = tp.tile([128, 4 * ND], BF, tag="tmp")
                    c_b = bass.AP(tensor=c_bf.tensor, offset=c_bf.offset,
                                  ap=[list(c_bf.ap[0]), [NCAP, 4], [0, D],
                                      [1, NCAP]])
                    nc.vector.tensor_mul(
                        w_g.rearrange("p (c d n) -> p c d n", c=4, d=D),
                        u_g.rearrange("p (c d n) -> p c d n", c=4, d=D),
                        c_b)
                    # s += sel_ch^T w   (accumulate over groups in PSUM)
                    for q in range(4):
                        ch = q & 1
                        for h in range(2):
                            nc.tensor.matmul(
                                s_ps[:, h * 512:(h + 1) * 512],
                                sel_sb[:, ch * 128:(ch + 1) * 128],
                                w_g[:, q * ND + h * 512:q * ND + (h + 1) * 512],
                                start=(g == 0 and q == 0),
                                stop=(g == SG - 1 and q == 3),
                                skip_group_check=True)

                pend = stage_a(0, vb)
                for g in range(1, SG):
                    nxt = stage_a(g, vb)
                    stage_b(g - 1, *pend)
                    pend = nxt
                stage_b(SG - 1, *pend)
                vb = reduce_squash_v(s_ps, 1.0, last=(it == 2))

    nc.compile()
    return nc


_CACHED = {}


def _get_program():
    if "nc" not in _CACHED:
        _CACHED["nc"] = _build_program()
    return _CACHED["nc"]


def kernel(x, W, bias):
    x = np.asarray(x, dtype=np.float32)
    W = np.asarray(W, dtype=np.float32)
    bias = np.asarray(bias, dtype=np.float32)

    w_all, xbd_all, xd_all, sels, bias_f = _host_prep(x, W, bias)
    nc = _get_program()

    in_maps = []
    for c in range(CORES):
        in_maps.append({
            "w_d": w_all[c],
            "xbd_d": xbd_all[c],
            "xd_d": xd_all[c],
            "sel_d": sels,
            "bias_d": bias_f,
        })
    res = run_bass_kernel_spmd(nc, in_maps, core_ids=list(range(CORES)))
    _CACHED["last_results"] = res
    # v_out is replicated; columns are (d,n) ordered -> [b, n, d]
    v = res.results[0]["v_out"].reshape(B, D, NCAP).transpose(0, 2, 1)
    return np.ascontiguousarray(v)


# revision 35
# speedup vs baseline: 1.0072x; 1.0072x over previous
"""ClassCapsule dynamic-routing kernel for 8 Trainium2 NeuronCores.

Problem (hardcoded shapes):
    x:    [64, 2048, 16]  fp32
    W:    [2048, 16, 1024] fp32
    bias: [64, 16]        fp32
    out:  [64, 64, 16]    fp32  (squeezed v after 3 routing iterations)

Strategy (in_caps-sharded, W resident in SBUF, u_hat recomputed per
iteration, per-iteration AllReduce of the small s tensor):
  - in_caps=2048 split across 8 cores (256 each); every core holds the
    full batch B=64.  W slice (bf16) lives in SBUF for the whole kernel,
    so u_hat is recomputed on the PE each routing iteration instead of
    being bounced through DRAM.  Total HBM traffic is ~12 MB/core.
  - u_hat tiles [128=(i4,b32), 1024=(d16,n64)] come from 4-way
    block-diagonal matmuls: lhsT [K=64=(i4,e16), M=128=(i4,b32)], two
    blocks packed across the 128 K-partitions (PE row strips at base
    partitions 0/64).  Column order (d major, n minor) keeps the
    free-dim broadcast of c packed so DVE runs in 2x bf16 mode.
  - iteration 0 (uniform c): s0 = sum_i u/64 collapses into a dense
    x^T @ W matmul over K=128 - no u_hat materialization at all.
  - routing: agreement = u*v reduced over d via a halving add tree
    (DVE, bf16), softmax over n (ACT exp + DVE), weighted sum over i
    via selector matmuls on the PE accumulating in PSUM.
  - s [64,1024] partials are AllReduced (collective_compute, bf16)
    across the 8 cores each iteration; squash/v computed redundantly.
"""

import numpy as np
import ml_dtypes

import concourse.bass as bass
import concourse.tile as tile
from concourse import bacc, mybir
from concourse.bass_utils import run_bass_kernel_spmd

# ---------------------------------------------------------------- constants
B, IC, E = 64, 2048, 16          # batch, in_caps, in_dim
NCAP, D = 64, 16                 # n_caps, cap_dim
ND = NCAP * D                    # 1024
CORES = 8
ICL = IC // CORES                # 256 local in_caps
SG = 32                          # column groups (8 in_caps each)
HB = 32                          # half-batch chunk
EPS = 1e-7

FP = mybir.dt.float32
BF = mybir.dt.bfloat16
BF_NP = ml_dtypes.bfloat16


def _host_prep(x, W, bias):
    """Per-core host-side tensors (bf16, (d,n) column order).

    Column group gg covers local in_caps i = 8*gg + 4*s + l  (strip s in
    {0,1} at partition base 64*s, lane l in 0..3).  Partition row layout
    for W / x lhsT: p = (s2, l4, e16)."""
    # W columns reordered from (n,d) to (d,n): new_col = d*64 + n
    W_dn = W.reshape(IC, E, NCAP, D).transpose(0, 1, 3, 2).reshape(IC, E, ND)

    w_all, xbd_all, xd_all = [], [], []
    for c in range(CORES):
        sl = slice(c * ICL, (c + 1) * ICL)
        W_c = W_dn[sl]                                   # [256, 16, 1024]
        # [gg, s, l, e, nd] -> [(s l e)=128, gg, nd]
        w_all.append(np.ascontiguousarray(
            W_c.reshape(SG, 2, 4, E, ND).transpose(1, 2, 3, 0, 4)
            .reshape(128, SG, ND)).astype(BF_NP))

        x_c = x[:, sl]                                   # [64, 256, 16]
        x_r = x_c.reshape(2, HB, SG, 2, 4, E).transpose(3, 4, 5, 2, 0, 1)
        # x_r: [s, l, e, gg, ch, b']
        # 4-way block-diag lhsT: [(s l e)=128, gg, ch, (l' b32)=128]
        arr = np.zeros((2, 4, E, SG, 2, 4, HB), dtype=np.float32)
        for l in range(4):
            arr[:, l, :, :, :, l, :] = x_r[:, l]
        xbd_all.append(arr.reshape(128, SG, 2, 128).astype(BF_NP))

        # dense lhsT for iter-0 direct sum: [(s l e)=128, gg, m=b pad 128]
        xd = np.zeros((128, SG, 128), dtype=np.float32)
        xd[:, :, :B] = x_c.reshape(B, SG, 2, 4, E).transpose(2, 3, 4, 1, 0) \
            .reshape(128, SG, B)
        xd_all.append(xd.astype(BF_NP))

    # selectors, one per batch chunk: sel[ch][k=(l4,b32), m=ch*32+(k%32)]
    sels = np.zeros((2, 128, 128), dtype=np.float32)
    for ch in range(2):
        k = np.arange(128)
        sels[ch, k, ch * HB + (k % HB)] = 1.0
    sels = np.ascontiguousarray(sels.transpose(1, 0, 2)).astype(BF_NP)

    # bias in (d,n) order, tiled over batch: [64, 1024]
    bias_dn = np.ascontiguousarray(bias.T).reshape(1, ND)       # [d,n] flat
    bias_f = np.tile(bias_dn, (B, 1)).astype(np.float32)
    return w_all, xbd_all, xd_all, sels, bias_f


def _build_program():
    nc = bacc.Bacc("TRN2", target_bir_lowering=False, num_devices=CORES)

    w_d = nc.dram_tensor("w_d", [128, SG, ND], BF, kind="ExternalInput")
    xbd_d = nc.dram_tensor("xbd_d", [128, SG, 2, 128], BF, kind="ExternalInput")
    xd_d = nc.dram_tensor("xd_d", [128, SG, 128], BF, kind="ExternalInput")
    sel_d = nc.dram_tensor("sel_d", [128, 2, 128], BF, kind="ExternalInput")
    bias_d = nc.dram_tensor("bias_d", [B, ND], FP, kind="ExternalInput")
    v_out = nc.dram_tensor("v_out", [B, ND], FP, kind="ExternalOutput")

    v_scr = nc.dram_tensor("v_scr", [B, ND], BF)     # bounce for vb build

    with tile.TileContext(nc) as tc:
        with (
            tc.tile_pool(name="consts", bufs=1) as cp,
            tc.tile_pool(name="ubf", bufs=5) as up,       # [128, 4096] grouped
            tc.tile_pool(name="tmp", bufs=3) as tp,       # [128, 4096] tmp+ws ring
            tc.tile_pool(name="vb", bufs=1) as vbp,
            tc.tile_pool(name="smalls", bufs=2) as sp,
            tc.tile_pool(name="sq", bufs=1) as qp,
            tc.tile_pool(name="ups", bufs=3, space="PSUM") as psp,
            tc.tile_pool(name="sps", bufs=1, space="PSUM") as psa,
            tc.tile_pool(name="bstate", bufs=1) as bsp,
            tc.tile_pool(name="dram", bufs=2, space="DRAM") as dp,
        ):
            # ---- resident tensors.  Load order matters: iter 0 needs xd +
            # w chunks; xbd is only needed at iter 1 so it loads last.
            xd_sb = up.tile([128, SG * 128], BF, tag="u_g")
            nc.sync.dma_start(out=xd_sb, in_=xd_d[:, :, :])
            sel_sb = cp.tile([128, 2 * 128], BF)
            nc.sync.dma_start(out=sel_sb, in_=sel_d[:, :, :])
            bias_sb = cp.tile([B, ND], FP)
            nc.sync.dma_start(out=bias_sb, in_=bias_d[:, :])
            eps_t = cp.tile([B, 1], FP)
            nc.vector.memset(eps_t, EPS)
            w_sb = cp.tile([128, SG * ND], BF)
            WCH = 4  # groups per load chunk; per-chunk deps let iter0 start early
            for chk in range(SG // WCH):
                nc.sync.dma_start(
                    out=w_sb[:, chk * WCH * ND:(chk + 1) * WCH * ND],
                    in_=w_d[:, chk * WCH:(chk + 1) * WCH, :])
            xbd_sb = cp.tile([128, SG * 2 * 128], BF)
            nc.sync.dma_start(out=xbd_sb, in_=xbd_d[:, :, :, :])

            # warm up the collective path while inputs stream in: the first
            # AllReduce pays one-time channel setup, so do a tiny dummy one
            warm_in = dp.tile([B, 4], FP, tag="warm_in")
            warm_out = dp.tile([B, 4], FP, tag="warm_out")
            warm_sb = cp.tile([B, 4], FP)
            nc.vector.memset(warm_sb, 0.0)
            nc.sync.dma_start(out=warm_in[:], in_=warm_sb)
            nc.gpsimd.collective_compute(
                "AllReduce",
                mybir.AluOpType.add,
                replica_groups=[list(range(CORES))],
                ins=[warm_in[:].opt()],
                outs=[warm_out[:].opt()],
            )

            # routing logits state: [128=(l4,b32), (gg, s, ch, n)]
            b_all = bsp.tile([128, SG * 4 * NCAP], FP)

            # ---------------- AllReduce s -> (scale,bias) -> squash -> v
            def reduce_squash_v(s_ps, scale, last):
                # AllReduce in bf16: halves the collective payload; the
                # ~0.4% rounding on s is well inside the error budget
                s_par = qp.tile([B, ND], BF, tag="q0")
                nc.scalar.copy(out=s_par, in_=s_ps[0:B, :])
                s_in = dp.tile([B, ND], BF, tag="cc_in")
                nc.sync.dma_start(out=s_in[:], in_=s_par)
                s_red = dp.tile([B, ND], BF, tag="cc_out")
                nc.gpsimd.collective_compute(
                    "AllReduce",
                    mybir.AluOpType.add,
                    replica_groups=[list(range(CORES))],
                    ins=[s_in[:].opt()],
                    outs=[s_red[:].opt()],
                )
                s_glob = qp.tile([B, ND], BF, tag="q2b")
                nc.sync.dma_start(out=s_glob, in_=s_red[:])
                # s = s_glob*scale + bias
                s_sb = qp.tile([B, ND], FP, tag="q1")
                nc.vector.scalar_tensor_tensor(
                    out=s_sb, in0=s_glob, scalar=float(scale), in1=bias_sb,
                    op0=mybir.AluOpType.mult, op1=mybir.AluOpType.add)
                sqr = qp.tile([B, ND], FP, tag="q2")
                nc.scalar.square(out=sqr, in_=s_sb)
                nsq = sp.tile([B, NCAP], FP, tag="nsq")
                nc.vector.reduce_sum(
                    out=nsq, in_=sqr.rearrange("p (d n) -> p n d", d=D),
                    axis=mybir.AxisListType.X)
                norm = sp.tile([B, NCAP], FP, tag="norm")
                nc.scalar.activation(out=norm, in_=nsq,
                                     func=mybir.ActivationFunctionType.Sqrt,
                                     bias=eps_t[:, :], scale=1.0)
                den = sp.tile([B, NCAP], FP, tag="den")
                nc.vector.scalar_tensor_tensor(
                    out=den, in0=nsq, scalar=float(EPS + 1.0), in1=norm,
                    op0=mybir.AluOpType.add, op1=mybir.AluOpType.mult)
                rden = sp.tile([B, NCAP], FP, tag="rden")
                nc.vector.reciprocal(out=rden, in_=den)
                fac = sp.tile([B, NCAP], FP, tag="fac")
                nc.vector.scalar_tensor_tensor(
                    out=fac, in0=nsq, scalar=float(EPS), in1=rden,
                    op0=mybir.AluOpType.add, op1=mybir.AluOpType.mult)
                v_sb = qp.tile([B, ND], FP, tag="q2")
                fac_b = bass.AP(tensor=fac.tensor, offset=fac.offset,
                                ap=[list(fac.ap[0]), [0, D], list(fac.ap[1])])
                nc.vector.tensor_mul(
                    v_sb.rearrange("p (d n) -> p d n", d=D),
                    s_sb.rearrange("p (d n) -> p d n", d=D),
                    fac_b)
                if last:
                    nc.sync.dma_start(out=v_out[:, :], in_=v_sb)
                    return None
                v_bf = qp.tile([B, ND], BF, tag="q0")
                nc.vector.tensor_copy(out=v_bf, in_=v_sb)
                nc.sync.dma_start(out=v_scr[:, :], in_=v_bf)
                # vb[128=(l4,b32), (ch,d,n)]: row (l,b') of ch holds v[ch*32+b']
                vb = vbp.tile([128, 2 * ND], BF, tag="vb")
                for ch in range(2):
                    src = bass.AP(tensor=v_scr, offset=ch * HB * ND,
                                  ap=[[0, 4], [ND, HB], [1, ND]])
                    nc.sync.dma_start(
                        out=vb[:, ch * ND:(ch + 1) * ND], in_=src)
                return vb

            # ================= iter 0: s0 = (1/64) sum_i u  ================
            s_ps = psa.tile([128, ND], FP, tag="s_acc")
            for g in range(SG):
                for h in range(2):
                    nc.tensor.matmul(
                        s_ps[:, h * 512:(h + 1) * 512],
                        xd_sb[:, g * 128:(g + 1) * 128],
                        w_sb[:, g * ND + h * 512:g * ND + (h + 1) * 512],
                        start=(g == 0), stop=(g == SG - 1))
            vb = reduce_squash_v(s_ps, 1.0 / NCAP, last=False)

            # ================= routing iterations 1 and 2 =================
            # Software-pipelined: stage A(g) = proj + psum copy + u*v +
            # d-reduce tree + exp; stage B(g) = Z/recip/scale + u*c + sel
            # matmuls.  Emitting A(g+1) before B(g) keeps the (in-order)
            # DVE queue fed while ACT's exp and GpSimd's scale for block g
            # are still in flight.
            for it in (1, 2):
                s_ps = psa.tile([128, ND], FP, tag="s_acc")

                def stage_a(g, vb, it=it):
                    u_g = up.tile([128, 4 * ND], BF, tag="u_g")
                    for q in range(4):          # q = (s strip, ch half-batch)
                        s_, ch = q >> 1, q & 1
                        u_ps = psp.tile([128, ND], FP, tag="u_ps")
                        lhs = xbd_sb[64 * s_:64 * (s_ + 1),
                                     (g * 2 + ch) * 128:(g * 2 + ch + 1) * 128]
                        for h in range(2):
                            nc.tensor.matmul(
                                u_ps[:, h * 512:(h + 1) * 512],
                                lhs,
                                w_sb[64 * s_:64 * (s_ + 1),
                                     g * ND + h * 512:g * ND + (h + 1) * 512],
                                start=True, stop=True)
                        nc.scalar.copy(
                            out=u_g[:, q * ND:(q + 1) * ND], in_=u_ps)
                    # tmp = u * v  (bf16, packed -> 2x DVE; v bcast over s)
                    tmp = tp.tile([128, 4 * ND], BF, tag="tmp")
                    vb_b = bass.AP(tensor=vb.tensor, offset=vb.offset,
                                   ap=[list(vb.ap[0]), [0, 2], [ND, 2],
                                       [1, ND]])
                    nc.vector.tensor_mul(
                        tmp.rearrange("p (s c f) -> p s c f", s=2, c=2),
                        u_g.rearrange("p (s c f) -> p s c f", s=2, c=2),
                        vb_b)
                    # reduce over d: halving add tree on [p, q, (d n)]
                    t3 = tmp.rearrange("p (c f) -> p c f", c=4)
                    b_dst = b_all[:, g * 4 * NCAP:(g + 1) * 4 * NCAP]
                    b3 = b_dst.rearrange("p (c n) -> p c n", c=4)
                    for half in (512, 256, 128, 64):
                        src_hi = bass.AP(
                            tensor=tmp.tensor, offset=tmp.offset + half,
                            ap=[list(tmp.ap[0]), [ND, 4], [1, half]])
                        if half > 64:
                            nc.vector.tensor_add(
                                t3[:, :, 0:half], t3[:, :, 0:half], src_hi)
                        else:
                            # final add -> b state (fp32, contiguous)
                            if it == 1:
                                nc.vector.tensor_add(
                                    b3, t3[:, :, 0:64], src_hi)
                            else:
                                agr = sp.tile([128, 4 * NCAP], FP, tag="agr")
                                a3 = agr.rearrange("p (c n) -> p c n", c=4)
                                nc.vector.tensor_add(
                                    a3, t3[:, :, 0:64], src_hi)
                                nc.vector.tensor_add(b_dst, b_dst, agr)
                    # softmax numerator: exp on ACT
                    c_un = sp.tile([128, 4 * NCAP], BF, tag="c_un")
                    nc.scalar.activation(
                        out=c_un, in_=b_dst,
                        func=mybir.ActivationFunctionType.Exp)
                    return u_g, c_un

                def stage_b(g, u_g, c_un):
                    zsum = sp.tile([128, 4], FP, tag="zsum")
                    nc.vector.reduce_sum(
                        out=zsum, in_=c_un.rearrange("p (c n) -> p c n", c=4),
                        axis=mybir.AxisListType.X)
                    rec = sp.tile([128, 4], BF, tag="rec")
                    with nc.allow_low_precision(reason="1/Z in bf16 is fine for softmax scale"):
                        nc.vector.reciprocal(out=rec, in_=zsum)
                    c_bf = sp.tile([128, 4 * NCAP], BF, tag="c_bf")
                    rec_b = bass.AP(tensor=rec.tensor, offset=rec.offset,
                                    ap=[list(rec.ap[0]), [1, 4], [0, NCAP]])
                    nc.gpsimd.tensor_mul(
                        c_bf.rearrange("p (c n) -> p c n", c=4),
                        c_un.rearrange("p (c n) -> p c n", c=4),
                        rec_b)
                    # w = u * c (c bcast over d; last dim packed -> 2x DVE)
                    w_g = tp.tile([128, 4 * ND], BF, tag="tmp")
                    c_b = bass.AP(tensor=c_bf.tensor, offset=c_bf.offset,
                                  ap=[list(c_bf.ap[0]), [NCAP, 4], [0, D],
                                      [1, NCAP]])
                    nc.vector.tensor_mul(
                        w_g.rearrange("p (c d n) -> p c d n", c=4, d=D),
                        u_g.rearrange("p (c d n) -> p c d n", c=4, d=D),
                        c_b)
                    # s += sel_ch^T w   (accumulate over groups in PSUM)
                    for q in range(4):
                        ch = q & 1
                        for h in range(2):
                            nc.tensor.matmul(
                                s_ps[:, h * 512:(h + 1) * 512],
                                sel_sb[:, ch * 128:(ch + 1) * 128],
                                w_g[:, q * ND + h * 512:q * ND + (h + 1) * 512],
                                start=(g == 0 and q == 0),
                                stop=(g == SG - 1 and q == 3),
                                skip_group_check=True)

                pend = stage_a(0, vb)
                for g in range(1, SG):
                    nxt = stage_a(g, vb)
                    stage_b(g - 1, *pend)
                    pend = nxt
                stage_b(SG - 1, *pend)
                vb = reduce_squash_v(s_ps, 1.0, last=(it == 2))

    nc.compile()
    return nc


_CACHED = {}


def _get_program():
    if "nc" not in _CACHED:
        _CACHED["nc"] = _build_program()
    return _CACHED["nc"]


def kernel(x, W, bias):
    x = np.asarray(x, dtype=np.float32)
    W = np.asarray(W, dtype=np.float32)
    bias = np.asarray(bias, dtype=np.float32)

    w_all, xbd_all, xd_all, sels, bias_f = _host_prep(x, W, bias)
    nc = _get_program()

    in_maps = []
    for c in range(CORES):
        in_maps.append({
            "w_d": w_all[c],
            "xbd_d": xbd_all[c],
            "xd_d": xd_all[c],
            "sel_d": sels,
            "bias_d": bias_f,
        })
    res = run_bass_kernel_spmd(nc, in_maps, core_ids=list(range(CORES)))
    _CACHED["last_results"] = res
    # v_out is replicated; columns are (d,n) ordered -> [b, n, d]
    v = res.results[0]["v_out"].reshape(B, D, NCAP).transpose(0, 2, 1)
    return np.ascontiguousarray(v)
